# revision 3
# baseline (speedup 1.0000x reference)
"""Trainium2 Bass kernel for CustomAttention (qkv -> per-head LN on q,k -> SDPA -> proj).

Sharding: 8 cores = 2 batches x 4 head-groups (3 heads each).
Per core: qkv projection for its heads from x[b], full attention per head
(scores computed transposed so no probability-matrix transpose is needed,
softmax denominator folded into the PV matmul as a ones column on V),
then a partial output projection over its 192 channels. Host sums the 4
partials per batch and adds proj_b.

Engine balance (vs naive):
 - LN stats via DVE bn_stats/bn_aggr, elementwise apply alternating
   vector/gpsimd per n-block so two LN chains run concurrently.
 - All PSUM->SBUF copies merged (one strided copy per q/k/vA group) and
   pinned to scalar/vector so neither engine saturates.
 - exp() calls widened to 1536 columns (3 psum banks) to amortize the
   ~300-cycle ACT fixed overhead; scalar engine does nothing else in the
   attention phase.
 - PV accumulator and proj psum share one rotating [128,512] psum tag so
   everything fits in the 8 psum banks.
"""

import os
import sys
from functools import lru_cache

import numpy as np

for _p in ("/opt/trn_rl_repo", os.path.expanduser("~/.axon_site/_ro/trn_rl_repo")):
    if os.path.isdir(_p) and _p not in sys.path:
        sys.path.insert(0, _p)

import concourse.bass as bass
import concourse.mybir as mybir
from concourse import bacc
import concourse.tile as tile
from concourse.masks import make_identity

F32 = mybir.dt.float32
F32R = mybir.dt.float32r
BF16 = mybir.dt.bfloat16
ALU = mybir.AluOpType
ACTF = mybir.ActivationFunctionType
AXL = mybir.AxisListType

H = 3          # heads per core
D = 64         # head dim
C = 768        # model dim
J = 3 * H * D  # qkv rows per core = 576
EPS = 1e-5
SCALE = D ** -0.5

SKEW_CHUNKS = 6   # PV matmuls lag the score/exp pipeline by this many j-chunks


def r32(ap):
    return ap.bitcast(F32R)


def build_nc(N=4096):
    """One-core program; all 8 cores run it SPMD with different input data."""
    NB = N // 128          # j-chunks / n-blocks
    IB = N // 512          # i-blocks

    nc = bacc.Bacc("TRN2", target_bir_lowering=False, debug=False)
    x_t = nc.declare_dram_parameter("x_t", [C, N], BF16, isOutput=False)
    wqkv_t = nc.declare_dram_parameter("wqkv_t", [C, J], BF16, isOutput=False)
    projw_t = nc.declare_dram_parameter("projw_t", [H * D, C], F32, isOutput=False)
    gb = nc.declare_dram_parameter("gb", [2, 2 * H * D], F32, isOutput=False)
    out_p = nc.declare_dram_parameter("out_p", [N, C], F32, isOutput=True)

    with tile.TileContext(nc) as tc:
        with (
            tc.tile_pool(name="persist", bufs=1) as persist,
            tc.tile_pool(name="weights", bufs=1) as weights,
        ):
            # ---- persistent SBUF tensors ----
            # qT duplicated across both partition halves: rows 0:64 == 64:128
            qT = persist.tile([128, H, N], BF16, tag="qT")
            # kT stacked: rows 0:64 = j in [0,N/2), rows 64:128 = j in [N/2,N)
            kT = persist.tile([128, H, N // 2], BF16, tag="kT")
            # V augmented with a ones column (index 64) per j-chunk
            vA = persist.tile([128, H, NB, 65], BF16, tag="vA")
            # attention output, channel-major: ao1 rows = h0,h1; ao2 rows = h2
            ao1 = persist.tile([128, N], F32R, tag="ao1")
            ao2 = persist.tile([64, N], F32R, tag="ao2")

            ident = persist.tile([128, 128], F32, tag="ident")
            make_identity(nc, ident[:])
            identb = persist.tile([128, 128], BF16, tag="identb")
            nc.vector.tensor_copy(identb[:], ident[:])
            nc.vector.memset(vA[:, :, :, 64:65], 1.0)

            wq = weights.tile([128, 6, J], BF16, tag="wqkv")
            nc.sync.dma_start(
                wq[:], wqkv_t.rearrange("(ck p) j -> p ck j", p=128)
            )
            pw128 = weights.tile([128, C], F32R, tag="pw128")
            nc.sync.dma_start(pw128[:], projw_t[0:128, :].bitcast(F32R))
            pw64 = weights.tile([64, C], F32R, tag="pw64")
            nc.sync.dma_start(pw64[:], projw_t[128:192, :].bitcast(F32R))
            # row 0 = gammas for 6 vheads (q*scale x3, k x3), row 1 = betas
            gbt = weights.tile([128, 2, 2 * H, D], F32, tag="gb")
            epst = weights.tile([128, 1], F32, tag="epst")
            nc.vector.memset(epst[:], EPS)
            nc.sync.dma_start(
                gbt[:],
                gb.rearrange("r (g d) -> r g d", d=D)[None, :, :, :].to_broadcast(
                    [128, 2, 2 * H, D]
                ),
            )

            # ================= Phase B: qkv + LN + transpose =================
            with (
                tc.tile_pool(name="pB", bufs=3) as pB,
                tc.tile_pool(name="pBs", bufs=4) as pBs,
                tc.tile_pool(name="psQ", bufs=3, space="PSUM") as psQ,
                tc.tile_pool(name="psT", bufs=2, space="PSUM") as psT,
            ):
                lnos = {}
                psts = {}

                def emit_qkv(nb):
                    xt = pB.tile([128, 6, 128], BF16, tag="xt")
                    nc.sync.dma_start(
                        xt[:],
                        x_t.rearrange("(ck p) n -> p ck n", p=128)[
                            :, :, nb * 128 : (nb + 1) * 128
                        ],
                    )
                    # q at cols 0:192, k at 192:384 (bank 0), v at 512:704 (bank 1)
                    ps = psQ.tile([128, 1024], F32, tag="qkvps")
                    for g, off in ((0, 0), (1, 192), (2, 512)):
                        for ck in range(6):
                            nc.tensor.matmul(
                                ps[:, off : off + 192],
                                xt[:, ck, :],
                                wq[:, ck, g * 192 : (g + 1) * 192],
                                start=(ck == 0),
                                stop=(ck == 5),
                            )
                    return ps

                def emit_ln(nb, ps):
                    # stage q|k to SBUF via scalar (gpsimd has no PSUM port)
                    qkvS = pBs.tile([128, 2 * H, D], F32, tag="qkvS", bufs=3)
                    nc.scalar.copy(
                        qkvS[:], ps[:, 0 : 2 * H * D].rearrange("p (g d) -> p g d", d=D)
                    )
                    src3 = qkvS[:]

                    s1 = pBs.tile([128, 2 * H], F32, tag="s1")
                    nc.vector.tensor_reduce(s1[:], src3, AXL.X, ALU.add)
                    sq = pBs.tile([128, 2 * H, D], F32, tag="sq")
                    nc.gpsimd.tensor_mul(sq[:], src3, src3)
                    s2 = pBs.tile([128, 2 * H], F32, tag="s2")
                    nc.vector.tensor_reduce(s2[:], sq[:], AXL.X, ALU.add)

                    mu = pBs.tile([128, 2 * H], F32, tag="mu")
                    nc.gpsimd.tensor_scalar_mul(mu[:], s1[:], 1.0 / D)
                    var = pBs.tile([128, 2 * H], F32, tag="var")
                    nc.gpsimd.tensor_scalar_mul(var[:], s2[:], 1.0 / D)
                    musq = pBs.tile([128, 2 * H], F32, tag="musq")
                    nc.gpsimd.tensor_mul(musq[:], mu[:], mu[:])
                    nc.gpsimd.tensor_sub(var[:], var[:], musq[:])

                    std = pBs.tile([128, 2 * H], F32, tag="std")
                    nc.scalar.activation(std[:], var[:], ACTF.Sqrt, bias=epst[:])
                    rstd = pBs.tile([128, 2 * H], F32, tag="rstd")
                    nc.vector.reciprocal(rstd[:], std[:])
                    # one Newton step on gpsimd: r <- r*(1.5 - 0.5*(var+eps)*r^2)
                    nr = pBs.tile([128, 2 * H], F32, tag="nr")
                    nc.gpsimd.tensor_mul(nr[:], rstd[:], rstd[:])
                    ve = pBs.tile([128, 2 * H], F32, tag="ve")
                    nc.gpsimd.tensor_scalar_add(ve[:], var[:], EPS)
                    nc.gpsimd.tensor_mul(nr[:], nr[:], ve[:])
                    nc.gpsimd.tensor_scalar(nr[:], nr[:], -0.5, 1.5, ALU.mult, ALU.add)
                    rstd2 = pBs.tile([128, 2 * H], F32, tag="rstd2")
                    nc.gpsimd.tensor_mul(rstd2[:], rstd[:], nr[:])

                    cs = pBs.tile([128, 2 * H, D], F32, tag="cs")
                    nc.gpsimd.tensor_sub(
                        cs[:], src3, mu[:, :, None].broadcast_to([128, 2 * H, D])
                    )
                    nc.gpsimd.tensor_mul(
                        cs[:], cs[:], rstd2[:, :, None].broadcast_to([128, 2 * H, D])
                    )
                    nc.vector.tensor_mul(cs[:], cs[:], gbt[:, 0, :, :])
                    # write q and k LN output duplicated over both transpose halves
                    lno = pBs.tile([128, 2 * H, 2, D], BF16, tag="lno")
                    nc.gpsimd.tensor_add(
                        lno[:],
                        cs[:, :, None, :].broadcast_to([128, 2 * H, 2, D]),
                        gbt[:, 1, :, None, :].broadcast_to([128, 2 * H, 2, D]),
                    )
                    # v slab -> vA (scalar, psum read)
                    nc.scalar.copy(
                        vA[:, :, nb, 0:64],
                        ps[:, 512 : 512 + H * D].rearrange("p (h d) -> p h d", d=D),
                    )
                    return lno

                def emit_transp(nb):
                    lno = lnos.pop(nb)
                    lnof = lno.rearrange("p g r d -> p (g r d)")
                    pst = psT.tile([128, 2 * H, 128], BF16, tag="pst")
                    for g in range(2 * H):
                        nc.tensor.transpose(
                            pst[:, g, :], lnof[:, g * 128 : (g + 1) * 128], identb[:]
                        )
                    psts[nb] = pst

                def emit_copies(nb):
                    pst = psts.pop(nb)
                    blk = slice(nb * 128, (nb + 1) * 128)
                    nc.scalar.copy(qT[:, :, blk], pst[:, 0:H, :])
                    jh = nb // (NB // 2)
                    cb = nb % (NB // 2)
                    rows = slice(64 * jh, 64 * jh + 64)
                    nc.vector.tensor_copy(
                        kT[rows, :, cb * 128 : (cb + 1) * 128], pst[rows, H : 2 * H, :]
                    )

                for slot in range(NB + 2):
                    if slot < NB:
                        ps = emit_qkv(slot)
                        lnos[slot] = emit_ln(slot, ps)
                    if slot >= 2:
                        emit_transp(slot - 2)
                        emit_copies(slot - 2)

            # ================= Phase C: attention + proj =================
            NT = (NB + 2) // 3          # score/exp tiles per (ib, h)
            with (
                tc.tile_pool(name="pt", bufs=4) as ptp,
                tc.tile_pool(name="pCs", bufs=4) as pCs,
                tc.tile_pool(name="pD", bufs=3) as pD,
                tc.tile_pool(name="psS", bufs=2, space="PSUM") as psS,
                tc.tile_pool(name="psOD", bufs=2, space="PSUM") as psOD,
            ):
                for ib in range(IB):
                    isl = slice(ib * 512, (ib + 1) * 512)
                    for h in range(H):
                        pso_t = psOD.tile([128, 512], F32, tag="psod", name="pso")
                        pso = pso_t[0:65, :]
                        queue = []
                        n_pv = [0]

                        def emit_pv(pso=pso, queue=queue, n_pv=n_pv, h=h):
                            pt_ap, jc = queue.pop(0)
                            nc.tensor.matmul(
                                pso,
                                vA[:, h, jc, :],
                                pt_ap,
                                start=(n_pv[0] == 0),
                                stop=(n_pv[0] == NB - 1),
                            )
                            n_pv[0] += 1

                        for t in range(NT):
                            nch = min(3, NB - 3 * t)
                            W = 512 * nch
                            ps = psS.tile([128, 1536], F32, tag="st")
                            for s in range(nch):
                                jc = 3 * t + s
                                p0 = 0 if jc < NB // 2 else 64
                                jf = (jc % (NB // 2)) * 128
                                nc.tensor.matmul(
                                    ps[:, 512 * s : 512 * s + 512],
                                    kT[p0 : p0 + 64, h, jf : jf + 128],
                                    qT[p0 : p0 + 64, h, isl],
                                    start=True,
                                    stop=True,
                                    tile_position=(p0, 0),
                                )
                            pt = ptp.tile([128, 1536], BF16, tag="pt")
                            nc.scalar.activation(pt[:, 0:W], ps[:, 0:W], ACTF.Exp)
                            for s in range(nch):
                                queue.append((pt[:, 512 * s : 512 * s + 512], 3 * t + s))
                            while len(queue) > SKEW_CHUNKS:
                                emit_pv()
                        while queue:
                            emit_pv()

                        rden_f = pCs.tile([1, 512], F32, tag="rden_f")
                        nc.vector.tensor_copy(rden_f[:], pso[64:65, :])
                        rden = pCs.tile([1, 512], F32, tag="rden")
                        nc.vector.reciprocal_approx_fast(rden[:], rden_f[:])
                        rb = pCs.tile([64, 512], F32, tag="rb")
                        nc.gpsimd.partition_broadcast(rb[:], rden[:])
                        if h == 0:
                            nc.vector.tensor_mul(ao1[0:64, isl], pso[0:64, :], rb[:])
                        elif h == 2:
                            nc.vector.tensor_mul(ao2[0:64, isl], pso[0:64, :], rb[:])
                        else:
                            stg = pCs.tile([64, 512], F32R, tag="stg")
                            nc.vector.tensor_mul(stg[:], pso[0:64, :], rb[:])
                            nc.sync.dma_start(ao1[64:128, isl], stg[:])
                    for nb in range(ib * 4, ib * 4 + 4):
                        blk = slice(nb * 128, (nb + 1) * 128)
                        stage = pD.tile([128, C], F32, tag="stage")
                        for oc, osz in ((0, 512), (512, 256)):
                            pd_t = psOD.tile([128, 512], F32, tag="psod", name="pd")
                            nc.tensor.matmul(
                                pd_t[:, 0:osz],
                                r32(ao1[:, blk]),
                                r32(pw128[:, oc : oc + osz]),
                                start=True,
                                stop=False,
                            )
                            nc.tensor.matmul(
                                pd_t[:, 0:osz],
                                r32(ao2[0:64, blk]),
                                r32(pw64[0:64, oc : oc + osz]),
                                start=False,
                                stop=True,
                            )
                            nc.vector.tensor_copy(stage[:, oc : oc + osz], pd_t[:, 0:osz])
                        nc.sync.dma_start(out_p[blk, :], stage[:])

    nc.compile()
    return nc


@lru_cache(maxsize=2)
def _built(N):
    nc = build_nc(N)
    return nc


def _prep_inputs(x, qkv_w, q_gamma, q_beta, k_gamma, k_beta, proj_w):
    x = np.asarray(x, np.float32)
    qkv_w = np.asarray(qkv_w, np.float32)
    proj_w = np.asarray(proj_w, np.float32)
    B = x.shape[0]
    import ml_dtypes
    xts = [np.ascontiguousarray(x[b].T).astype(ml_dtypes.bfloat16) for b in range(B)]
    gq = np.tile(np.asarray(q_gamma, np.float32) * SCALE, H)
    bq = np.tile(np.asarray(q_beta, np.float32) * SCALE, H)
    gk = np.tile(np.asarray(k_gamma, np.float32), H)
    bk = np.tile(np.asarray(k_beta, np.float32), H)
    gb2 = np.stack([np.concatenate([gq, gk]), np.concatenate([bq, bk])])
    gbs = []
    wqs = []
    pws = []
    for g in range(4):
        r = slice(192 * g, 192 * (g + 1))
        wq_rows = np.concatenate(
            [qkv_w[r], qkv_w[768:1536][r], qkv_w[1536:2304][r]], axis=0
        )
        wqs.append(np.ascontiguousarray(wq_rows.T).astype(ml_dtypes.bfloat16))
        pws.append(np.ascontiguousarray(proj_w[:, r].T))
        gbs.append(gb2)
    in_maps = []
    for core in range(8):
        b, g = core // 4, core % 4
        in_maps.append(
            {"x_t": xts[b], "wqkv_t": wqs[g], "projw_t": pws[g], "gb": gbs[g]}
        )
    return in_maps


def run_cores(in_maps, N, trace=False):
    from concourse.bass_utils import run_bass_kernel_spmd

    nc = _built(N)
    res = run_bass_kernel_spmd(nc, in_maps, list(range(8)), trace=trace)
    return res


def kernel(x, qkv_w, q_gamma, q_beta, k_gamma, k_beta, proj_w, proj_b):
    x = np.asarray(x, np.float32)
    N = x.shape[1]
    in_maps = _prep_inputs(x, qkv_w, q_gamma, q_beta, k_gamma, k_beta, proj_w)
    res = run_cores(in_maps, N)
    parts = [np.asarray(r["out_p"], np.float32) for r in res.results]
    out0 = parts[0] + parts[1] + parts[2] + parts[3]
    out1 = parts[4] + parts[5] + parts[6] + parts[7]
    out = np.stack([out0, out1]) + np.asarray(proj_b, np.float32)
    return out.astype(np.float32)


# revision 4
# speedup vs baseline: 1.4340x; 1.4340x over previous
"""Trainium2 Bass kernel for CustomAttention (qkv -> per-head LN on q,k -> SDPA -> proj).

Sharding: 8 cores = 2 batches x 4 head-groups (3 heads each).
Per core: qkv projection for its heads from x[b], full attention per head
(scores computed transposed so no probability-matrix transpose is needed,
softmax denominator folded into the PV matmul as a ones column on V),
then a partial output projection over its 192 channels. Host sums the 4
partials per batch and adds proj_b.

Schedule notes:
 - Phase B is software-pipelined 4 slots deep so each engine's in-order
   queue only ever sees work whose inputs are already computed.
 - LN merged q|k (6 virtual heads, one 384-wide chain); rstd computed as
   exp(-0.5*ln(var+eps)) on the scalar engine (same ACT table set as the
   attention exp, so no table reloads).
 - Attention scores alternate PE quadrants (tile_position row 0/64) every
   matmul so adjacent score matmuls execute concurrently; PV matmuls are
   interleaved one-per-score to keep the array busy.
 - exp() widened to 1536 columns to amortize ACT fixed overhead; scalar
   does nothing else in phase C.
"""

import os
import sys
from functools import lru_cache

import numpy as np

for _p in ("/opt/trn_rl_repo", os.path.expanduser("~/.axon_site/_ro/trn_rl_repo")):
    if os.path.isdir(_p) and _p not in sys.path:
        sys.path.insert(0, _p)

import concourse.bass as bass
import concourse.mybir as mybir
from concourse import bacc
import concourse.tile as tile
from concourse.masks import make_identity

F32 = mybir.dt.float32
F32R = mybir.dt.float32r
BF16 = mybir.dt.bfloat16
ALU = mybir.AluOpType
ACTF = mybir.ActivationFunctionType
AXL = mybir.AxisListType

H = 3          # heads per core
D = 64         # head dim
C = 768        # model dim
J = 3 * H * D  # qkv rows per core = 576
G = 2 * H      # merged LN virtual heads (q0..2, k0..2)
EPS = 1e-5
SCALE = D ** -0.5

SKEW_CHUNKS = 8   # PV matmuls lag the score/exp pipeline by this many j-chunks


def r32(ap):
    return ap.bitcast(F32R)


def build_nc(N=4096):
    """One-core program; all 8 cores run it SPMD with different input data."""
    NB = N // 128          # j-chunks / n-blocks
    IB = N // 512          # i-blocks

    nc = bacc.Bacc("TRN2", target_bir_lowering=False, debug=False)
    x_t = nc.declare_dram_parameter("x_t", [C, N], BF16, isOutput=False)
    wqkv_t = nc.declare_dram_parameter("wqkv_t", [C, J], BF16, isOutput=False)
    projw_t = nc.declare_dram_parameter("projw_t", [H * D, C], F32, isOutput=False)
    gb = nc.declare_dram_parameter("gb", [2, G * D], F32, isOutput=False)
    out_p = nc.declare_dram_parameter("out_p", [N, C], F32, isOutput=True)

    with tile.TileContext(nc) as tc:
        with (
            tc.tile_pool(name="persist", bufs=1) as persist,
            tc.tile_pool(name="weights", bufs=1) as weights,
        ):
            # ---- persistent SBUF tensors ----
            # qT duplicated across both partition halves: rows 0:64 == 64:128
            qT = persist.tile([128, H, N], BF16, tag="qT")
            # kT stacked: rows 0:64 = j in [0,N/2), rows 64:128 = j in [N/2,N)
            kT = persist.tile([128, H, N // 2], BF16, tag="kT")
            # V augmented with a ones column (index 64) per j-chunk
            vA = persist.tile([128, H, NB, 65], BF16, tag="vA")
            # attention output, channel-major: ao1 rows = h0,h1; ao2 rows = h2
            ao1 = persist.tile([128, N], F32R, tag="ao1")
            ao2 = persist.tile([64, N], F32R, tag="ao2")

            ident = persist.tile([128, 128], F32, tag="ident")
            make_identity(nc, ident[:])
            identb = persist.tile([128, 128], BF16, tag="identb")
            nc.vector.tensor_copy(identb[:], ident[:])
            nc.vector.memset(vA[:, :, :, 64:65], 1.0)

            wq = weights.tile([128, 6, J], BF16, tag="wqkv")
            nc.sync.dma_start(
                wq[:], wqkv_t.rearrange("(ck p) j -> p ck j", p=128)
            )
            pw128 = weights.tile([128, C], F32R, tag="pw128")
            nc.sync.dma_start(pw128[:], projw_t[0:128, :].bitcast(F32R))
            pw64 = weights.tile([64, C], F32R, tag="pw64")
            nc.sync.dma_start(pw64[:], projw_t[128:192, :].bitcast(F32R))
            # row 0 = gammas for 6 vheads (q*scale x3, k x3), row 1 = betas
            gbt = weights.tile([128, 2, G, D], F32, tag="gb")
            epst = weights.tile([128, 1], F32, tag="epst")
            nc.vector.memset(epst[:], EPS)
            nc.sync.dma_start(
                gbt[:],
                gb.rearrange("r (g d) -> r g d", d=D)[None, :, :, :].to_broadcast(
                    [128, 2, G, D]
                ),
            )

            # ================= Phase B: qkv + LN + transpose =================
            # software pipeline, slot s handles: qkv(s), stats(s-1), apply(s-2),
            # transpose+copies(s-3)
            with (
                tc.tile_pool(name="pB", bufs=3) as pB,
                tc.tile_pool(name="pBs", bufs=3) as pBs,
                tc.tile_pool(name="psQ", bufs=3, space="PSUM") as psQ,
                tc.tile_pool(name="psT", bufs=2, space="PSUM") as psT,
            ):
                st = {}   # per-nb dict of live tiles

                def emit_qkv(nb):
                    xt = pB.tile([128, 6, 128], BF16, tag="xt")
                    nc.sync.dma_start(
                        xt[:],
                        x_t.rearrange("(ck p) n -> p ck n", p=128)[
                            :, :, nb * 128 : (nb + 1) * 128
                        ],
                    )
                    # q at cols 0:192, k at 192:384 (bank 0), v at 512:704 (bank 1)
                    ps = psQ.tile([128, 1024], F32, tag="qkvps")
                    for g, off in ((0, 0), (1, 192), (2, 512)):
                        for ck in range(6):
                            nc.tensor.matmul(
                                ps[:, off : off + 192],
                                xt[:, ck, :],
                                wq[:, ck, g * 192 : (g + 1) * 192],
                                start=(ck == 0),
                                stop=(ck == 5),
                            )
                    st[nb] = {"ps": ps}

                def emit_stats(nb):
                    t = st[nb]
                    ps = t.pop("ps")
                    qkvS = pBs.tile([128, G, D], F32, tag="qkvS")
                    nc.scalar.copy(
                        qkvS[:], ps[:, 0 : G * D].rearrange("p (g d) -> p g d", d=D)
                    )
                    nc.scalar.copy(
                        vA[:, :, nb, 0:64],
                        ps[:, 512 : 512 + H * D].rearrange("p (h d) -> p h d", d=D),
                    )
                    s1 = pBs.tile([128, G], F32, tag="s1")
                    nc.vector.tensor_reduce(s1[:], qkvS[:], AXL.X, ALU.add)
                    mu = pBs.tile([128, G], F32, tag="mu")
                    nc.vector.tensor_scalar_mul(mu[:], s1[:], 1.0 / D)
                    sq = pBs.tile([128, G, D], F32, tag="sq")
                    nc.gpsimd.tensor_mul(sq[:], qkvS[:], qkvS[:])
                    s2 = pBs.tile([128, G], F32, tag="s2")
                    nc.vector.tensor_reduce(s2[:], sq[:], AXL.X, ALU.add)
                    musq = pBs.tile([128, G], F32, tag="musq")
                    nc.vector.tensor_mul(musq[:], mu[:], mu[:])
                    var = pBs.tile([128, G], F32, tag="var")
                    nc.vector.scalar_tensor_tensor(
                        var[:], s2[:], 1.0 / D, musq[:], ALU.mult, ALU.subtract
                    )
                    t.update(qkvS=qkvS, mu=mu, var=var)

                def emit_apply(nb):
                    t = st[nb]
                    qkvS, mu, var = t.pop("qkvS"), t.pop("mu"), t.pop("var")
                    # rstd = exp(-0.5 * ln(var + eps)) -- both in the same ACT
                    # table set as phase C's exp, so no table reloads anywhere.
                    lnv = pBs.tile([128, G], F32, tag="lnv")
                    nc.scalar.activation(lnv[:], var[:], ACTF.Ln, bias=epst[:])
                    rstd = pBs.tile([128, G], F32, tag="rstd")
                    nc.scalar.activation(rstd[:], lnv[:], ACTF.Exp, scale=-0.5)
                    # cs = (x - mu) * rstd, fused per vhead on vector
                    cs = pBs.tile([128, G, D], F32, tag="cs")
                    for g in range(G):
                        nc.vector.tensor_scalar(
                            cs[:, g, :],
                            qkvS[:, g, :],
                            mu[:, g : g + 1],
                            rstd[:, g : g + 1],
                            ALU.subtract,
                            ALU.mult,
                        )
                    csg = pBs.tile([128, G, D], F32, tag="csg")
                    nc.gpsimd.tensor_mul(csg[:], cs[:], gbt[:, 0, :, :])
                    lno = pBs.tile([128, G, 2, D], BF16, tag="lno")
                    nc.gpsimd.tensor_add(
                        lno[:],
                        csg[:, :, None, :].broadcast_to([128, G, 2, D]),
                        gbt[:, 1, :, None, :].broadcast_to([128, G, 2, D]),
                    )
                    t["lno"] = lno

                def emit_transp(nb):
                    t = st[nb]
                    lno = t.pop("lno")
                    lnof = lno.rearrange("p g r d -> p (g r d)")
                    pst = psT.tile([128, G, 128], BF16, tag="pst")
                    for g in range(G):
                        nc.tensor.transpose(
                            pst[:, g, :], lnof[:, g * 128 : (g + 1) * 128], identb[:]
                        )
                    blk = slice(nb * 128, (nb + 1) * 128)
                    nc.scalar.copy(qT[:, :, blk], pst[:, 0:H, :])
                    jh = nb // (NB // 2)
                    cb = nb % (NB // 2)
                    rows = slice(64 * jh, 64 * jh + 64)
                    nc.vector.tensor_copy(
                        kT[rows, :, cb * 128 : (cb + 1) * 128], pst[rows, H:G, :]
                    )
                    del st[nb]

                for s in range(NB + 3):
                    if s >= 2:
                        if s - 2 < NB:
                            emit_apply(s - 2)
                    if s < NB:
                        emit_qkv(s)
                    if s >= 1 and s - 1 < NB:
                        emit_stats(s - 1)
                    if s >= 3:
                        emit_transp(s - 3)

            # ================= Phase C: attention + proj =================
            NT = (NB + 2) // 3          # score/exp tiles per (ib, h)
            with (
                tc.tile_pool(name="pt", bufs=5) as ptp,
                tc.tile_pool(name="pCs", bufs=4) as pCs,
                tc.tile_pool(name="pD", bufs=3) as pD,
                tc.tile_pool(name="psS", bufs=2, space="PSUM") as psS,
                tc.tile_pool(name="psOD", bufs=2, space="PSUM") as psOD,
            ):
                # chunk order alternates PE quadrants so adjacent score
                # matmuls run concurrently: 0,16,1,17,2,18,...
                corder = []
                for i in range(NB // 2):
                    corder.append(i)
                    corder.append(i + NB // 2)

                for ib in range(IB):
                    isl = slice(ib * 512, (ib + 1) * 512)
                    for h in range(H):
                        pso_t = psOD.tile([128, 512], F32, tag="psod", name="pso")
                        pso = pso_t[0:65, :]
                        queue = []
                        n_pv = [0]

                        def emit_pv(pso=pso, queue=queue, n_pv=n_pv, h=h):
                            pt_ap, jc = queue.pop(0)
                            nc.tensor.matmul(
                                pso,
                                vA[:, h, jc, :],
                                pt_ap,
                                start=(n_pv[0] == 0),
                                stop=(n_pv[0] == NB - 1),
                            )
                            n_pv[0] += 1

                        for t in range(NT):
                            nch = min(3, NB - 3 * t)
                            W = 512 * nch
                            ps = psS.tile([128, 1536], F32, tag="st")
                            for s in range(nch):
                                jc = corder[3 * t + s]
                                p0 = 0 if jc < NB // 2 else 64
                                jf = (jc % (NB // 2)) * 128
                                nc.tensor.matmul(
                                    ps[:, 512 * s : 512 * s + 512],
                                    kT[p0 : p0 + 64, h, jf : jf + 128],
                                    qT[p0 : p0 + 64, h, isl],
                                    start=True,
                                    stop=True,
                                    tile_position=(p0, 0),
                                )
                                if len(queue) > SKEW_CHUNKS:
                                    emit_pv()
                            pt = ptp.tile([128, 1536], BF16, tag="pt")
                            nc.scalar.activation(pt[:, 0:W], ps[:, 0:W], ACTF.Exp)
                            for s in range(nch):
                                queue.append(
                                    (pt[:, 512 * s : 512 * s + 512], corder[3 * t + s])
                                )
                        while queue:
                            emit_pv()

                        rden_f = pCs.tile([1, 512], F32, tag="rden_f")
                        nc.vector.tensor_copy(rden_f[:], pso[64:65, :])
                        rden = pCs.tile([1, 512], F32, tag="rden")
                        nc.vector.reciprocal_approx_fast(rden[:], rden_f[:])
                        rb = pCs.tile([64, 512], F32, tag="rb")
                        nc.gpsimd.partition_broadcast(rb[:], rden[:])
                        if h == 0:
                            nc.vector.tensor_mul(ao1[0:64, isl], pso[0:64, :], rb[:])
                        elif h == 2:
                            nc.vector.tensor_mul(ao2[0:64, isl], pso[0:64, :], rb[:])
                        else:
                            stg = pCs.tile([64, 512], F32R, tag="stg")
                            nc.vector.tensor_mul(stg[:], pso[0:64, :], rb[:])
                            nc.sync.dma_start(ao1[64:128, isl], stg[:])
                    for nb in range(ib * 4, ib * 4 + 4):
                        blk = slice(nb * 128, (nb + 1) * 128)
                        stage = pD.tile([128, C], F32, tag="stage")
                        for oc, osz in ((0, 512), (512, 256)):
                            pd_t = psOD.tile([128, 512], F32, tag="psod", name="pd")
                            nc.tensor.matmul(
                                pd_t[:, 0:osz],
                                r32(ao1[:, blk]),
                                r32(pw128[:, oc : oc + osz]),
                                start=True,
                                stop=False,
                            )
                            nc.tensor.matmul(
                                pd_t[:, 0:osz],
                                r32(ao2[0:64, blk]),
                                r32(pw64[0:64, oc : oc + osz]),
                                start=False,
                                stop=True,
                            )
                            nc.vector.tensor_copy(stage[:, oc : oc + osz], pd_t[:, 0:osz])
                        nc.sync.dma_start(out_p[blk, :], stage[:])

    nc.compile()
    return nc


@lru_cache(maxsize=2)
def _built(N):
    nc = build_nc(N)
    return nc


def _prep_inputs(x, qkv_w, q_gamma, q_beta, k_gamma, k_beta, proj_w):
    x = np.asarray(x, np.float32)
    qkv_w = np.asarray(qkv_w, np.float32)
    proj_w = np.asarray(proj_w, np.float32)
    B = x.shape[0]
    import ml_dtypes
    xts = [np.ascontiguousarray(x[b].T).astype(ml_dtypes.bfloat16) for b in range(B)]
    gq = np.tile(np.asarray(q_gamma, np.float32) * SCALE, H)
    bq = np.tile(np.asarray(q_beta, np.float32) * SCALE, H)
    gk = np.tile(np.asarray(k_gamma, np.float32), H)
    bk = np.tile(np.asarray(k_beta, np.float32), H)
    gb2 = np.stack([np.concatenate([gq, gk]), np.concatenate([bq, bk])])
    gbs = []
    wqs = []
    pws = []
    for g in range(4):
        r = slice(192 * g, 192 * (g + 1))
        wq_rows = np.concatenate(
            [qkv_w[r], qkv_w[768:1536][r], qkv_w[1536:2304][r]], axis=0
        )
        wqs.append(np.ascontiguousarray(wq_rows.T).astype(ml_dtypes.bfloat16))
        pws.append(np.ascontiguousarray(proj_w[:, r].T))
        gbs.append(gb2)
    in_maps = []
    for core in range(8):
        b, g = core // 4, core % 4
        in_maps.append(
            {"x_t": xts[b], "wqkv_t": wqs[g], "projw_t": pws[g], "gb": gbs[g]}
        )
    return in_maps


def run_cores(in_maps, N, trace=False):
    from concourse.bass_utils import run_bass_kernel_spmd

    nc = _built(N)
    res = run_bass_kernel_spmd(nc, in_maps, list(range(8)), trace=trace)
    return res


def kernel(x, qkv_w, q_gamma, q_beta, k_gamma, k_beta, proj_w, proj_b):
    x = np.asarray(x, np.float32)
    N = x.shape[1]
    in_maps = _prep_inputs(x, qkv_w, q_gamma, q_beta, k_gamma, k_beta, proj_w)
    res = run_cores(in_maps, N)
    parts = [np.asarray(r["out_p"], np.float32) for r in res.results]
    out0 = parts[0] + parts[1] + parts[2] + parts[3]
    out1 = parts[4] + parts[5] + parts[6] + parts[7]
    out = np.stack([out0, out1]) + np.asarray(proj_b, np.float32)
    return out.astype(np.float32)
